# revision 2
# baseline (speedup 1.0000x reference)
"""Trainium2 Bass kernel for nn_ExpansionContrastModule — v2 (fp8-DR convs).

Sharding: 8 cores = 4 batches x 2 H-halves (80 rows each), bottom halves
row-flipped on host (conv weights H-flipped to match). Within a core the 80
owned rows split into halves A/B on SBUF partitions 0:64 / 64:128.

v2 changes vs baseline:
- branch convs (and in_conv) run in fp8e4m3 with DoubleRow perf mode: two
  taps per matmul (vertical tap pairs, stride WP0; horizontal pairs stride 1).
  Weights are scaled x16 at quantization; 1/16 folded into the PSUM copy-out.
- all data DMAs issue from the SP queue (HWDGE) instead of gpsimd (SWDGE),
  freeing the Pool engine to take part of the sort network.
- the per-rank scale-sum (y = sum_j s'_j * sorted_j) runs on the PE as four
  accumulating diag-matmuls per 400-col chunk; Act copies PSUM->SBUF bf16.
- sort comparators are split DVE/Pool per a static assignment.
- final stage reworked into 400-col chunks: mt = vmax + 0.25*vsum via PE,
  BN+SiLU via Act/DVE, mask+1 broadcast via a 3-row matmul trick, final
  cen*(mask+1) as bf16 TT muls; outputs stored bf16, host casts to f32.
"""

import os

os.environ.setdefault("MYCRO_LOCAL_CACHE", "1")

import numpy as np
import ml_dtypes

import concourse.bass as bass
import concourse.bacc as bacc
import concourse.mybir as mybir
from concourse.tile import TileContext
from concourse import bass_utils

W = 160
SH = 100          # shard rows
HALO = 10
OWNH = 40         # owned rows per half
C = 256
CH = 64           # trunk channels
RB = 10           # post-stage block rows
CK = 400          # scale/final chunk cols (PSUM f32 1600B <= 1 bank)
KS = [1, 3, 5, 7]
DIL = [1, 3, 5, 7]
WP0 = W + 6       # x0 width, real cols at [3, 163)
X0R = 60
RT = 3            # conv tile rows (N=480)
WSCALE = 16.0     # fp8 weight pre-scale

F32 = mybir.dt.float32
BF16 = mybir.dt.bfloat16
F8 = mybir.dt.float8e4
ALU = mybir.AluOpType
ACTF = mybir.ActivationFunctionType
MMPM = mybir.MatmulPerfMode

NPF8 = ml_dtypes.float8_e4m3fn

# sort network: (dst, a, b, op); D/P engine assignment string (tunable)
SORT_NET = [
    ("e1", "t0", "t1", "max"), ("t0", "t0", "t1", "min"),
    ("t1", "t2", "t3", "max"), ("t2", "t2", "t3", "min"),
    ("t3", "t0", "t2", "min"), ("t0", "t0", "t2", "max"),
    ("t2", "e1", "t1", "min"), ("e1", "e1", "t1", "max"),
    ("t1", "t0", "t2", "min"), ("t0", "t0", "t2", "max"),
]
SORT_ENG = "DPDPPPDDDP"  # per-op engine: D=DVE, P=Pool (tune)
# sorted ascending: j=0 -> t3, 1 -> t1, 2 -> t0, 3 -> e1
SORTED_TILES = ["t3", "t1", "t0", "e1"]


def dr_pairs(ksz):
    """DoubleRow tap pairs [(kiA,kjA,kiB,kjB)]; B None => zero pad slab.

    All pairs use slab stride +WP0 (vertical): the runtime rejects small
    slab strides (stride 7 fails, 166 works). Last-row taps go in alone
    with a zeroed second weight slab that overreads one row (x0 has one
    extra allocated, zeroed row to keep the read in bounds).
    """
    ps = []
    for ki in range(0, ksz - 1, 2):
        for kj in range(ksz):
            ps.append((ki, kj, ki + 1, kj))
    for kj in range(ksz):
        ps.append((ksz - 1, kj, None, None))
    return ps


NPAIR = [len(dr_pairs(k)) for k in KS]          # [1, 5, 13, 25]
PAIR_OFF = [sum(NPAIR[:i]) for i in range(4)]   # cumulative
NPAIR_TOT = sum(NPAIR)                          # 44


def build_nc():
    nc = bacc.Bacc("TRN2", target_bir_lowering=False, debug=False,
                   enable_asserts=False, num_devices=8)

    def dram(name, shape, dt, kind="ExternalInput"):
        return nc.dram_tensor(name, list(shape), dt, kind=kind).ap()

    cen8 = dram("cen8", (128, 2 * SH * W), F8)       # c-groups concatenated
    cenb = dram("cenb", (128, 2 * 2 * OWNH * W), BF16)
    win = dram("win", (128, 128), F8)            # in_conv DR weights [ci,(s m)]
    wtap = dram("wtap", (128, NPAIR_TOT * 256), F8)
    sdg = dram("sdg", (128, 22 * 128), BF16)     # 5 diag mats x 4 k + I, -I
    wbc = dram("wbc", (128, 128), BF16)
    wfc = dram("wfc", (128, 2), BF16)
    wmt = dram("wmt", (128, 2 * 128), BF16)      # [I; 0.25 I]
    wmb = dram("wmb", (2, 2 * 128), BF16)        # mask broadcast lhsTs
    one1 = dram("one1", (128, 1), F32)           # +1 bias for m1s
    bin_ = dram("bin", (128, 1), F32)
    cb = dram("cb", (128, 4), F32)
    bnsc = dram("bnsc", (128, 1), F32)
    bnbi = dram("bnbi", (128, 1), F32)
    fcb = dram("fcb", (2, 1), F32)
    out_d = dram("out", (128, 2 * 2 * OWNH * W), BF16, kind="ExternalOutput")

    with TileContext(nc) as tc:
        with tc.tile_pool(name="cpool", bufs=1) as cp, \
             tc.tile_pool(name="inpool", bufs=2) as ip, \
             tc.tile_pool(name="x0pool", bufs=1) as x0p, \
             tc.tile_pool(name="xpool", bufs=2) as xp, \
             tc.tile_pool(name="o1pool", bufs=2) as o1p, \
             tc.tile_pool(name="tpool", bufs=2) as tp, \
             tc.tile_pool(name="ypool", bufs=2) as yp, \
             tc.tile_pool(name="vpool", bufs=1) as vp, \
             tc.tile_pool(name="fpool", bufs=2) as fp, \
             tc.tile_pool(name="pspool", bufs=1, space="PSUM") as pp:

            # load only in_conv-critical constants before the cen tiles;
            # the rest queue on SP after the first ct loads (SP FIFO is
            # in-order, so early big weight loads would delay the pipeline)
            win_s = cp.tile_from(win, name="win_s")
            bin_s = cp.tile_from(bin_, name="bin_s")

            # ---- Phase A: in_conv -> x0 [128, 61*166] fp8 (61st row stays
            # zero: overread target for pad-slab DR matmuls)
            x0 = x0p.tile([128, (X0R + 1) * WP0], F8, name="x0")
            # zero only pad regions (in_conv writes the rest): row-0 left
            # pad; right pad of each row + left pad of next (strided); the
            # tail of overread row 60; A-half image-pad rows.
            nc.gpsimd.memset(x0[:, 0:3], 0.0)
            nc.gpsimd.memset(
                x0[:, 163:163 + X0R * WP0]
                .rearrange("p (r w) -> p r w", w=WP0)[:, :, 0:6], 0.0)
            nc.gpsimd.memset(x0[:, X0R * WP0 + 3:(X0R + 1) * WP0], 0.0)
            GR = 4 * RT  # rows per ct load (4 PSUM tiles)
            for t0 in range(0, X0R, GR):
                n = RT * W
                cts = {}
                for h, base in ((0, 0), (1, 40)):
                    ct = ip.tile([128, 2 * GR * W], F8, tag=f"ct{h}",
                                 name=f"ct{h}")
                    src = bass.AP(cen8.tensor,
                                  cen8.offset + (base + t0) * W,
                                  [list(cen8.ap[0]), [SH * W, 2],
                                   [1, GR * W]])
                    nc.sync.dma_start(out=ct[:, :], in_=src)
                    cts[h] = ct
                for sub in range(2):
                    ps = pp.tile([128, n], F32, tag="cvps", bufs=2,
                                 name="ps_in")
                    for h in range(2):
                        ct = cts[h]
                        for s in range(2):
                            nc.tensor.matmul(
                                ps[h * 64:h * 64 + 64, :],
                                lhsT=win_s[:, s * 64:(s + 1) * 64],
                                rhs=ct[:, s * GR * W + sub * n:
                                       s * GR * W + sub * n + n],
                                start=(s == 0), stop=(s == 1))
                    nc.scalar.activation(
                        x0[:, :].rearrange("p (r w) -> p r w", w=WP0)
                        [:, t0 + sub * RT:t0 + sub * RT + RT, 3:3 + W],
                        ps[:, :].rearrange("p (r w) -> p r w", w=W),
                        ACTF.Identity, bias=bin_s[:, 0:1],
                        scale=1.0 / WSCALE)
            nc.gpsimd.memset(x0[0:64, 0:HALO * WP0], 0.0)

            # ---- vmax / vsum accumulators in ONE tile [128, 2*40*160] bf16
            # (one tile so the final-stage DoubleRow matmul can read both as
            # slabs of a single AP with stride OWNH*W)
            vboth = vp.tile([128, 2 * OWNH * W], BF16, name="vboth")
            vmax = vboth[:, 0:OWNH * W]
            vsum = vboth[:, OWNH * W:2 * OWNH * W]

            x0t = x0[:, :]
            pdim = list(x0t.ap[0])

            wtap_s = cp.tile_from(wtap, name="wtap_s")
            cb_s = cp.tile_from(cb, name="cb_s")
            sdg_s = cp.tile_from(sdg, name="sdg_s")
            wbc_s = cp.tile_from(wbc, name="wbc_s")
            wfc_s = cp.tile_from(wfc, name="wfc_s")
            wmt_s = cp.tile_from(wmt, name="wmt_s")
            wmb_s = cp.tile_from(wmb, name="wmb_s")
            bnsc_s = cp.tile_from(bnsc, name="bnsc_s")
            bnbi_s = cp.tile_from(bnbi, name="bnbi_s")
            fcb_s = cp.tile_from(fcb, name="fcb_s")
            one1_s = cp.tile_from(one1, name="one1_s")

            GF = 1600  # = RB*W: one final group per k=3 post block

            def emit_final_group(g):
                cent = fp.tile([128, 4 * GF], BF16, tag="cent", bufs=1,
                               name="cent")
                nc.sync.dma_start(
                    out=cent[:, :],
                    in_=bass.AP(cenb.tensor, cenb.offset + g * GF,
                                [list(cenb.ap[0]), [2 * OWNH * W, 2],
                                 [OWNH * W, 2], [1, GF]]))
                obig = fp.tile([128, 4 * GF], BF16, tag="obig", bufs=1,
                               name="obig")
                for uu in range(GF // CK):
                    o = g * GF + uu * CK
                    mt = pp.tile([128, CK], F32, tag="mt", bufs=1, name="mt")
                    nc.tensor.matmul(mt[:, :], lhsT=wmt_s[:, 0:128],
                                     rhs=vmax[:, o:o + CK], start=True,
                                     stop=False)
                    nc.tensor.matmul(mt[:, :], lhsT=wmt_s[:, 128:256],
                                     rhs=vsum[:, o:o + CK], start=False,
                                     stop=True)
                    mr = fp.tile([128, CK], BF16, tag="mr", name="mr")
                    nc.scalar.activation(mr[:, :], mt[:, :], ACTF.Relu)
                    zps = pp.tile([128, CK], F32, tag="zps", bufs=1,
                                  name="zps")
                    nc.tensor.matmul(zps[:, :], lhsT=wbc_s[:, :],
                                     rhs=mr[:, :], start=True, stop=True)
                    zt = fp.tile([128, CK], BF16, tag="zt", name="zt")
                    nc.scalar.activation(zt[:, :], zps[:, :], ACTF.Silu,
                                         bias=bnbi_s[:, 0:1],
                                         scale=bnsc_s[:, 0:1])
                    lps = pp.tile([2, CK], F32, tag="lps", bufs=1, name="lps")
                    nc.tensor.matmul(lps[:, :], lhsT=wfc_s[:, :],
                                     rhs=zt[:, :], start=True, stop=True)
                    # sigmoid via tanh (keeps every Act func in the
                    # silu_and_others table): sig(x) = 0.5 + 0.5*tanh(x/2);
                    # the 0.5 factor folds into wmb, the +1.5 into one1.
                    msk = fp.tile([2, CK], BF16, tag="msk", name="msk")
                    nc.scalar.activation(msk[:, :], lps[:, :], ACTF.Tanh,
                                         bias=fcb_s[:, 0:1], scale=0.5)
                    for h in range(2):
                        mb1 = pp.tile([128, CK], F32, tag="mb1", bufs=1,
                                      name="mb1")
                        nc.tensor.matmul(
                            mb1[:, :], lhsT=wmb_s[:, h * 128:(h + 1) * 128],
                            rhs=msk[:, :], start=True, stop=True)
                        m1s = fp.tile([128, CK], BF16, tag=f"m1s{h}",
                                      name="m1s")
                        nc.scalar.activation(m1s[:, :], mb1[:, :],
                                             ACTF.Identity,
                                             bias=one1_s[:, 0:1])
                        for c in range(2):
                            sl = slice((2 * c + h) * GF + uu * CK,
                                       (2 * c + h) * GF + uu * CK + CK)
                            nc.vector.tensor_mul(obig[:, sl], cent[:, sl],
                                                 m1s[:, :])
                nc.gpsimd.dma_start(
                    out=bass.AP(out_d.tensor, out_d.offset + g * GF,
                                [list(out_d.ap[0]), [2 * OWNH * W, 2],
                                 [OWNH * W, 2], [1, GF]]),
                    in_=obig[:, :])

            for k in range(4):
                d = DIL[k]
                ksz = KS[k]
                pad = ksz // 2
                rows_x = OWNH + 2 * d
                Wx = W + 4 * d
                xk = xp.tile([128, rows_x * Wx], BF16, tag="x", name=f"x{k}")
                xv = xk[:, :].rearrange("p (r w) -> p r w", w=Wx)
                nc.gpsimd.memset(xk[:, 0:2 * d], 0.0)
                nc.gpsimd.memset(
                    xk[:, 2 * d + W:2 * d + W + (rows_x - 1) * Wx]
                    .rearrange("p (r w) -> p r w", w=Wx)[:, :, 0:4 * d], 0.0)
                nc.gpsimd.memset(
                    xk[:, (rows_x - 1) * Wx + 2 * d + W:rows_x * Wx], 0.0)

                pairs = dr_pairs(ksz)
                for rt in range(0, rows_x, RT):
                    nr = min(RT, rows_x - rt)
                    n = nr * W
                    ps = pp.tile([128, n], F32, tag="cvps", bufs=2,
                                 name=f"ps{k}")
                    for pi, (kiA, kjA, kiB, kjB) in enumerate(pairs):
                        dyA, dxA = kiA - pad, kjA - pad
                        offA = (HALO - d + rt + dyA) * WP0 + 3 + dxA
                        if kiB is None:
                            delta = WP0  # zero weight slab; in-bounds overread
                        else:
                            delta = (kiB - kiA) * WP0 + (kjB - kjA)
                        rhs = bass.AP(x0t.tensor, x0t.offset + offA,
                                      [pdim, [delta, 2], [WP0, nr], [1, W]])
                        pcol = (PAIR_OFF[k] + pi) * 256
                        lhsT = wtap_s[:, pcol:pcol + 256].rearrange(
                            "p (two m) -> p two m", two=2)
                        nc.tensor.matmul(ps[:, :], lhsT=lhsT, rhs=rhs,
                                         start=(pi == 0),
                                         stop=(pi == len(pairs) - 1),
                                         perf_mode=MMPM.DoubleRow)
                    nc.scalar.activation(
                        xv[:, rt:rt + nr, 2 * d:2 * d + W],
                        ps[:, :].rearrange("p (r w) -> p r w", w=W),
                        ACTF.Identity, bias=cb_s[:, k:k + 1],
                        scale=1.0 / WSCALE)
                nc.gpsimd.memset(xk[0:64, 0:d * Wx], 0.0)

                # ---- post stage: blocks of RB owned rows
                WPK = W + 2 * d
                vs = [(-d, -d), (-d, 0), (-d, d), (0, -d)]
                for b in range(0, OWNH, RB):
                    tt = {}
                    for j, (dy, dx) in enumerate(vs):
                        # tight o1 region: only rows/cols both product
                        # factors touch
                        rows_j = RB + (d if dy else 0)
                        cols_j = W + (d if dx else 0)
                        cstart = d if dx > 0 else 2 * d
                        o1 = o1p.tile([128, rows_j * cols_j], BF16, tag="o1",
                                      bufs=1, name=f"o1_{k}_{b}_{j}")
                        o1v = o1[:, :].rearrange("p (r w) -> p r w", w=cols_j)
                        xr0 = d + b
                        nc.vector.tensor_sub(
                            o1v[:, :, :],
                            xv[:, xr0:xr0 + rows_j, cstart:cstart + cols_j],
                            xv[:, xr0 + dy:xr0 + dy + rows_j,
                               cstart + dx:cstart + dx + cols_j])
                        tj = tp.tile([128, RB * W], BF16, tag=f"t{j}",
                                     bufs=1, name=f"t{k}_{b}_{j}")
                        f1c = 2 * d - cstart
                        f2c = 2 * d - dx - cstart
                        nc.vector.tensor_mul(
                            tj[:, :].rearrange("p (r w) -> p r w", w=W),
                            o1v[:, 0:RB, f1c:f1c + W],
                            o1v[:, -dy:-dy + RB, f2c:f2c + W])
                        tt[f"t{j}"] = tj
                    tt["e1"] = tp.tile([128, RB * W], BF16, tag="e1",
                                       bufs=1, name=f"e{k}_{b}")
                    tt["v6"] = tp.tile([128, RB * W], BF16, tag="v6",
                                       bufs=1, name=f"v{k}_{b}")
                    # lean sort: A=e1, C=t0, B=t1, D=t2, u=t3, v=v6;
                    # o4=A+B-u, o1=C+D-v, o2+o3=u+v, o3-o2=|u-v| are folded
                    # into the PE scale matmuls (+ one Act Abs for |u-v|).
                    for dst, a, bb_, op in [
                            ("e1", "t0", "t1", "max"),
                            ("t0", "t0", "t1", "min"),
                            ("t1", "t2", "t3", "max"),
                            ("t2", "t2", "t3", "min"),
                            ("t3", "e1", "t1", "min"),
                            ("v6", "t0", "t2", "max")]:
                        nc.vector.tensor_tensor(
                            tt[dst][:, :], tt[a][:, :], tt[bb_][:, :],
                            ALU.max if op == "max" else ALU.min)
                    ybig = yp.tile([128, RB * W], BF16, tag="y",
                                   name=f"y{k}_{b}")
                    YT = [("e1", 0), ("t1", 0), ("t0", 1), ("t2", 1),
                          ("t3", 2), ("v6", 3)]
                    for uc in range(0, RB * W, CK):
                        psw = pp.tile([128, CK], F32, tag="ypsum", bufs=2,
                                      name=f"psw{k}_{b}")
                        nc.tensor.matmul(psw[:, :],
                                         lhsT=sdg_s[:, 20 * 128:21 * 128],
                                         rhs=tt["t3"][:, uc:uc + CK],
                                         start=True, stop=False)
                        nc.tensor.matmul(psw[:, :],
                                         lhsT=sdg_s[:, 21 * 128:22 * 128],
                                         rhs=tt["v6"][:, uc:uc + CK],
                                         start=False, stop=True)
                        wab = yp.tile([128, CK], BF16, tag="wab", bufs=2,
                                      name=f"wab{k}_{b}")
                        nc.scalar.activation(wab[:, :], psw[:, :], ACTF.Abs)
                        psy = pp.tile([128, CK], F32, tag="ypsum", bufs=2,
                                      name=f"psy{k}_{b}")
                        for ti, (tn, ci) in enumerate(YT):
                            scol = (k * 5 + ci) * 128
                            nc.tensor.matmul(
                                psy[:, :],
                                lhsT=sdg_s[:, scol:scol + 128],
                                rhs=tt[tn][:, uc:uc + CK],
                                start=(ti == 0), stop=False)
                        nc.tensor.matmul(
                            psy[:, :],
                            lhsT=sdg_s[:, (k * 5 + 4) * 128:
                                       (k * 5 + 5) * 128],
                            rhs=wab[:, :], start=False, stop=True)
                        nc.scalar.activation(ybig[:, uc:uc + CK], psy[:, :],
                                             ACTF.Identity)
                    vmx = vmax[:, b * W:(b + RB) * W]
                    vsm = vsum[:, b * W:(b + RB) * W]
                    if k == 0:
                        nc.vector.tensor_copy(vmx, ybig[:, :])
                        nc.vector.tensor_copy(vsm, ybig[:, :])
                    else:
                        nc.vector.tensor_tensor(vmx, vmx, ybig[:, :],
                                                ALU.max)
                        nc.vector.tensor_tensor(vsm, vsm, ybig[:, :],
                                                ALU.add)
                    if k == 3:
                        emit_final_group(b // RB)

            # (final-stage groups are emitted inside the k==3 post loop via
            # emit_final_group so they overlap the tail of the k-loop)
    nc.compile()
    nc.finalize()
    return nc


_NC_CACHE = None


def _get_nc():
    global _NC_CACHE
    if _NC_CACHE is None:
        _NC_CACHE = build_nc()
    return _NC_CACHE


def _prep_core_inputs(cen_b, flip, wts):
    """cen_b: (256, 160, 160) f32 for this batch; flip: bottom half?"""
    (w_in, b_in, convs, sadj, bc_w, bn_scale, bn_bias, fc_w, fc_b) = wts
    lo = (1 if flip else 0) * 80 - HALO
    sh = np.zeros((C, SH, W), np.float32)
    r0, r1 = max(0, lo), min(160, lo + SH)
    sh[:, r0 - lo:r1 - lo] = cen_b[:, r0:r1]
    if flip:
        sh = sh[:, ::-1]
    sh = np.ascontiguousarray(sh)

    bf = ml_dtypes.bfloat16

    wtap = np.zeros((128, NPAIR_TOT * 256), NPF8)
    for k in range(4):
        ksz = KS[k]
        cw = convs[k][0]
        if flip:
            cw = cw[:, :, ::-1, :]
        cw8 = (cw * WSCALE).astype(NPF8)
        for pi, (kiA, kjA, kiB, kjB) in enumerate(dr_pairs(ksz)):
            pcol = (PAIR_OFF[k] + pi) * 256
            for s, (ki, kj) in enumerate(((kiA, kjA), (kiB, kjB))):
                if ki is None:
                    continue
                blk = cw8[:, :, ki, kj].T  # [ci, co]
                wtap[0:64, pcol + s * 128:pcol + s * 128 + 64] = blk
                wtap[64:128, pcol + s * 128 + 64:pcol + s * 128 + 128] = blk

    # in_conv DR weights [128, (2 slabs x 64 out)]: slab s contracts cen
    # channel group s (ct columns s*n..); same lhsT serves both halves.
    win = np.zeros((128, 128), NPF8)
    w8 = (w_in * WSCALE).astype(NPF8)  # (64, 256)
    for s in range(2):
        win[:, s * 64:(s + 1) * 64] = w8[:, s * 128:(s + 1) * 128].T

    wbc = np.zeros((128, 128), bf)
    wbc[0:64, 0:64] = bc_w.T.astype(bf)
    wbc[64:128, 64:128] = bc_w.T.astype(bf)

    wfc = np.zeros((128, 2), bf)
    wfc[0:64, 0] = fc_w.astype(bf)
    wfc[64:128, 1] = fc_w.astype(bf)

    sdg = np.zeros((128, 22 * 128), bf)
    ar = np.arange(128)
    for k in range(4):
        s1, s2, s3, s4 = [np.concatenate([sadj[:, k, j]] * 2)
                          for j in range(4)]
        al = (s2 + s3) / 2
        for i, cv in enumerate((s4, s1, al - s4, al - s1, (s3 - s2) / 2)):
            col = (k * 5 + i) * 128
            sdg[ar, col + ar] = cv.astype(bf)
    sdg[ar, 20 * 128 + ar] = 1.0
    sdg[ar, 21 * 128 + ar] = -1.0

    wmt = np.zeros((128, 2 * 128), bf)
    wmt[ar, ar] = 1.0
    wmt[ar, 128 + ar] = 0.25

    wmb = np.zeros((2, 2 * 128), bf)
    wmb[0, 0:128] = 0.5    # half A: broadcast 0.5*tanh_A
    wmb[1, 128:256] = 0.5  # half B: broadcast 0.5*tanh_B

    dup = lambda v: np.concatenate([v, v]).astype(np.float32).reshape(128, -1)
    flat = sh.reshape(C, SH * W)
    own = sh[:, HALO:HALO + 2 * OWNH].reshape(C, 2 * OWNH * W)
    m = {
        "cen8": np.concatenate([flat[0:128], flat[128:256]], 1).astype(NPF8),
        "cenb": np.concatenate([own[0:128], own[128:256]], 1).astype(bf),
        "win": win,
        "wtap": wtap,
        "sdg": sdg,
        "wbc": wbc,
        "wfc": wfc,
        "wmt": wmt,
        "wmb": wmb,
        "bin": dup(b_in),
        "cb": np.concatenate([np.stack([cbv for _, cbv in convs], 1)] * 2, 0)
              .astype(np.float32),
        "bnsc": dup(bn_scale),
        "bnbi": dup(bn_bias),
        "fcb": np.full((2, 1), fc_b * 0.5, np.float32),
        "one1": np.full((128, 1), 1.5, np.float32),
    }
    return m


def make_in_maps(inputs):
    cen = np.asarray(inputs["cen"], np.float32)
    w_in = np.asarray(inputs["in_conv_w"], np.float32).reshape(CH, C)
    convs = [(np.asarray(inputs[f"conv{k}_w"], np.float32),
              np.asarray(inputs[f"conv{k}_b"], np.float32))
             for k in (1, 3, 5, 7)]
    # t = o1(p)*o1(p-v) = -o_ref; sort(o)_j = -sort(t)_{3-j}
    sadj = -np.asarray(inputs["scales3"], np.float32)[:, ::-1]    # (64, 4)
    sadj4 = np.repeat(sadj[:, None, :], 4, axis=1)                # (64, 4, 4)
    bn_scale = (np.asarray(inputs["bn_gamma"]) /
                np.sqrt(np.asarray(inputs["bn_var"]) + 1e-5)).astype(np.float32)
    bn_bias = (np.asarray(inputs["bn_beta"]) -
               np.asarray(inputs["bn_mean"]) * bn_scale).astype(np.float32)
    wts = (w_in, np.asarray(inputs["in_conv_b"], np.float32), convs, sadj4,
           np.asarray(inputs["bc_w"], np.float32).reshape(CH, CH),
           bn_scale, bn_bias,
           np.asarray(inputs["fc_w"], np.float32).reshape(CH),
           float(np.asarray(inputs["fc_b"])[0]))
    in_maps = []
    for core in range(8):
        b, half = core // 2, core % 2
        in_maps.append(_prep_core_inputs(cen[b], half == 1, wts))
    return in_maps


def kernel(**inputs):
    in_maps = make_in_maps(inputs)
    nc = _get_nc()
    res = bass_utils.run_bass_kernel_spmd(nc, in_maps,
                                          core_ids=list(range(8)))
    out = np.empty((4, C, 160, W), np.float32)
    for core in range(8):
        b, half = core // 2, core % 2
        arr = res.results[core]["out"].astype(np.float32)
        # [p, c, h, r, w] -> [c*128+p, h*40+r, w]
        o = arr.reshape(128, 2, 2 * OWNH, W).transpose(1, 0, 2, 3) \
            .reshape(C, 2 * OWNH, W)
        if half == 1:
            o = o[:, ::-1]
        out[b, :, half * 80:(half + 1) * 80] = o
    return out
